# revision 1
# baseline (speedup 1.0000x reference)
"""Causal self-attention (B=4, T=2048, C=1024, 16 heads) on 8 Trainium2 cores.

Sharding: batch x head-group.  Core c handles batch b = c//2 and head group
hg = c%2 (8 heads = 4 head-pairs).  Each core computes q/k/v projections for
its heads, causal flash-style attention, and a partial output projection
(its 512 y-channels x full w_proj columns).  The host sums the two partials
per batch (tensor-parallel reduce done host-side at gather).

Per-core kernel (matmul operands in fp16, fp32 PSUM accumulation; fp16 was
measured at 4.6e-4 max rel err vs the fp32 reference, nearly matching fp32r's
2.4e-4 while running faster):
  Interleaved per t-tile j (one flat pool scope, so the scheduler overlaps
  the PE-bound projections with the ACT-bound attention):
    B(j): q/k/v projections for tile j from one streamed x pass -> qT/kT
          strips [128 (pair ch), t]; V transposed on the PE into V_wide
          strips (a ones-column rides along for the softmax denominator)
    C(j): attention (needs only B iters 0..j): ST = K@Q^T row-packed head
          pair; exp on ACT (PSUM->SBUF, causal additive mask on the diagonal
          128-strip only, fully-masked columns never computed); y^T
          accumulated in PSUM with the softmax denominator landing in rows
          64 (head A) / 0 (head B); normalized on eviction via DMA-broadcast
          reciprocal; then the output projection for tile j.
  PSUM split (8 banks): qkv-accum 1 + transpose 1 + scores/proj 4 + y 2.
"""
import numpy as np

import concourse.bass as bass
import concourse.tile as tile
from concourse import mybir, bacc
from concourse.bass_utils import run_bass_kernel_spmd

f32 = mybir.dt.float32
f32r = mybir.dt.float32r
Exp = mybir.ActivationFunctionType.Exp

B, T, C = 4, 2048, 1024
N_HEAD = 16
D = C // N_HEAD                 # 64
HPC = N_HEAD // 2               # heads per core = 8
NPAIR = HPC // 2                # head pairs per core = 4
CO_Q = C // 2                   # q channels per core = 512
CT = C // 128                   # contraction tiles for qkv = 8
TJ = T // 512                   # t super-tiles = 4
NS = T // 128                   # s tiles = 16
SCALE = float(D) ** -0.5        # 0.125
NEG = -1.0e30

_CACHE = {}

# build options (A/B benchable): rounds_engine: which engine does the
# f32->f32r rounding copies; dma_spread: issue input DMAs round-robin
# across HWDGE engines instead of all on sync.
OPTS = {"rounds_engine": "vector", "dma_spread": False, "mmdtype": "float16",
        "skip_b": False, "skip_c": False}


def _build_nc(reps=1):
    import contextlib
    from concourse.masks import make_identity

    nc = bacc.Bacc("TRN2", target_bir_lowering=False, debug=False)
    xT_d = nc.dram_tensor("xT", [C, T], f32, kind="ExternalInput").ap()
    wqT_d = nc.dram_tensor("wqT", [C, CO_Q], f32, kind="ExternalInput").ap()
    wkT_d = nc.dram_tensor("wkT", [C, CO_Q], f32, kind="ExternalInput").ap()
    wvT_d = nc.dram_tensor("wvT", [C, CO_Q], f32, kind="ExternalInput").ap()
    wpT_d = nc.dram_tensor("wpT", [CO_Q, C], f32, kind="ExternalInput").ap()
    mask_d = nc.dram_tensor("mask", [128, 128], f32, kind="ExternalInput").ap()
    out_d = nc.dram_tensor("out", [T, C], f32, kind="ExternalOutput").ap()
    rscr_d = nc.dram_tensor("rscr", [NPAIR, TJ, 2, 512], f32, kind="Internal").ap()

    with tile.TileContext(nc) as tc:
        for _rep in range(reps):
            _build_body(nc, tc, xT_d, wqT_d, wkT_d, wvT_d, wpT_d, mask_d,
                        out_d, rscr_d)

    nc.compile()
    return nc


def _build_body(nc, tc, xT_d, wqT_d, wkT_d, wvT_d, wpT_d, mask_d, out_d, rscr_d):
    import contextlib
    from concourse.masks import make_identity

    rnd = getattr(nc, OPTS["rounds_engine"])   # engine for rounding/cast copies
    MMD = getattr(mybir.dt, OPTS["mmdtype"])   # matmul operand dtype

    def dma(out, in_):
        nc.sync.dma_start(out=out, in_=in_)

    # All pools open in ONE scope: phase C shares no SBUF addresses with
    # phase B, so the scheduler can start attention (ACT-bound) while the
    # projections (PE-bound) are still streaming -- no phase serialization.
    with contextlib.ExitStack() as ctx:
        ep = ctx.enter_context
        persist = ep(tc.tile_pool(name="persist", bufs=1))
        wallp = ep(tc.tile_pool(name="wall", bufs=1))
        stg1 = ep(tc.tile_pool(name="stg1", bufs=4))
        xin1 = ep(tc.tile_pool(name="xin1", bufs=3))
        vstg = ep(tc.tile_pool(name="vstg", bufs=3))
        wp2 = ep(tc.tile_pool(name="wp2", bufs=1))
        pw = ep(tc.tile_pool(name="pw", bufs=4))
        yb = ep(tc.tile_pool(name="yb", bufs=3))
        ob = ep(tc.tile_pool(name="ob", bufs=3))
        rbp = ep(tc.tile_pool(name="rbp", bufs=3))
        bps1 = ep(tc.tile_pool(name="bps1", bufs=1, space="PSUM"))
        tps = ep(tc.tile_pool(name="tps", bufs=1, space="PSUM"))
        yps_pool = ep(tc.tile_pool(name="yps", bufs=1, space="PSUM"))
        sps = ep(tc.tile_pool(name="sps", bufs=4, space="PSUM"))
        pps = sps   # proj psum shares the st slots

        qT = persist.tile([128, NPAIR, TJ, 512], MMD)
        kT = persist.tile([128, NPAIR, TJ, 512], MMD)
        vA = persist.tile([128, NPAIR, NS, 128], MMD)
        vB = persist.tile([128, NPAIR, NS, 128], MMD)
        mask = persist.tile([128, 128], f32)
        dma(out=mask[:], in_=mask_d[:, :])
        onecol = persist.tile([128, 64], f32)
        nc.vector.memset(onecol[:], 0.0)
        nc.vector.memset(onecol[:, 0:1], 1.0)
        tmpl_src = bass.AP(tensor=onecol.tensor, offset=onecol.offset,
                           ap=[onecol.ap[0], [0, NPAIR], [0, NS], onecol.ap[1]])
        rnd.tensor_copy(vA[:, :, :, 64:128], tmpl_src)
        rnd.tensor_copy(vB[:, :, :, 0:64], tmpl_src)

        ident = wallp.tile([128, 128], f32)
        make_identity(nc, ident)

        # ---- phase B: q/k/v projections + V_wide (single x pass) ----
        w_r = wallp.tile([128, CT, 3, CO_Q], MMD)
        for i, wd in enumerate((wqT_d, wkT_d, wvT_d)):
            for ct in range(CT):
                wstg = stg1.tile([128, CO_Q], f32, tag="stg")
                dma(out=wstg[:], in_=wd[ct * 128:(ct + 1) * 128, :])
                rnd.tensor_copy(w_r[:, ct, i, :], wstg[:])
        wpT_r = wp2.tile([128, NPAIR, C], MMD)
        for p in range(NPAIR):
            wstg2 = stg1.tile([128, C], f32, tag="wstg2")
            dma(out=wstg2[:], in_=wpT_d[p * 128:(p + 1) * 128, :])
            rnd.tensor_copy(wpT_r[:, p, :], wstg2[:])

        for j in range(TJ):
            xr = xin1.tile([128, CT, 512], MMD, tag="xr")
            for ct in range(CT):
                xstg = stg1.tile([128, 512], f32, tag="stg")
                dma(out=xstg[:],
                    in_=xT_d[ct * 128:(ct + 1) * 128, j * 512:(j + 1) * 512])
                rnd.tensor_copy(xr[:, ct, :], xstg[:])
            for i, dst in ((0, qT), (1, kT)):
                for p in range(NPAIR):
                    ps = bps1.tile([128, 512], f32, tag="qkps")
                    for ct in range(CT):
                        nc.tensor.matmul(
                            ps[:], w_r[:, ct, i, p * 128:(p + 1) * 128],
                            xr[:, ct, :],
                            start=(ct == 0), stop=(ct == CT - 1))
                    nc.vector.tensor_copy(dst[:, p, j, :], ps[:])
            for p in range(NPAIR):
                ps = bps1.tile([128, 512], f32, tag="qkps")
                for ct in range(CT):
                    nc.tensor.matmul(
                        ps[:], w_r[:, ct, 2, p * 128:(p + 1) * 128], xr[:, ct, :],
                        start=(ct == 0), stop=(ct == CT - 1))
                vtmp = vstg.tile([128, 512], f32, tag="vtmp")
                nc.vector.tensor_copy(vtmp[:], ps[:])
                for sj in range(4):
                    si = j * 4 + sj
                    trp = tps.tile([128, 128], f32, tag="trp")
                    nc.tensor.transpose(
                        trp[:], vtmp[:, sj * 128:(sj + 1) * 128], ident[:])
                    nc.vector.tensor_copy(vA[:, p, si, 0:64], trp[:, 0:64])
                    nc.vector.tensor_copy(vB[:, p, si, 64:128], trp[:, 64:128])

            # ---- attention + projection for tile j (deps: B iters 0..j) ----
            Y = yb.tile([128, NPAIR, 512], MMD, tag="Y")
            for p in range(NPAIR):
                ypsA = yps_pool.tile([128, 512], f32, tag="ypsA")
                ypsB = yps_pool.tile([128, 512], f32, tag="ypsB")
                nsj = 4 * (j + 1)
                for si in range(nsj):
                    rel = si * 128 - j * 512
                    lo = max(rel, 0)
                    stA = sps.tile([128, 512], f32, tag="st")
                    stB = sps.tile([128, 512], f32, tag="st")
                    ko, ks = si // 4, (si % 4) * 128
                    nc.tensor.matmul(
                        stA[:, lo:], kT[0:64, p, ko, ks:ks + 128],
                        qT[0:64, p, j, lo:], start=True, stop=True)
                    nc.tensor.matmul(
                        stB[:, lo:], kT[64:128, p, ko, ks:ks + 128],
                        qT[64:128, p, j, lo:], start=True, stop=True)
                    if rel >= 0:   # diagonal block: additive causal mask
                        nc.vector.tensor_add(
                            stA[:, lo:lo + 128], stA[:, lo:lo + 128], mask[:])
                        nc.vector.tensor_add(
                            stB[:, lo:lo + 128], stB[:, lo:lo + 128], mask[:])
                    pA = pw.tile([128, 512], MMD, tag="pA")
                    pB = pw.tile([128, 512], MMD, tag="pB")
                    nc.scalar.activation(pA[:, lo:], stA[:, lo:], Exp, scale=SCALE)
                    nc.scalar.activation(pB[:, lo:], stB[:, lo:], Exp, scale=SCALE)
                    st = (si == 0)
                    sp = (si == nsj - 1)
                    nc.tensor.matmul(ypsA[:, lo:], vA[:, p, si, :], pA[:, lo:],
                                     start=st, stop=sp)
                    nc.tensor.matmul(ypsB[:, lo:], vB[:, p, si, :], pB[:, lo:],
                                     start=st, stop=sp)
                # normalize on eviction: lA at ypsA row 64, lB at ypsB row 0
                r = rbp.tile([64, 512], f32, tag="r")
                nc.vector.reciprocal(r[0:1, :], ypsA[64:65, :])
                nc.vector.reciprocal(r[32:33, :], ypsB[0:1, :])
                nc.sync.dma_start(out=rscr_d[p, j, 0:1, :], in_=r[0:1, :])
                nc.sync.dma_start(out=rscr_d[p, j, 1:2, :], in_=r[32:33, :])
                rb = rbp.tile([128, 512], f32, tag="rb")
                nc.sync.dma_start(
                    out=rb[0:64, :],
                    in_=rscr_d[p, j, 0:1, :].partition_broadcast(64))
                nc.sync.dma_start(
                    out=rb[64:128, :],
                    in_=rscr_d[p, j, 1:2, :].partition_broadcast(64))
                nc.vector.tensor_mul(Y[0:64, p, :], ypsA[0:64, :], rb[0:64, :])
                nc.vector.tensor_mul(Y[64:128, p, :], ypsB[64:128, :],
                                     rb[64:128, :])

            # output projection for this t super-tile
            for tj in range(4):
                o_sb = ob.tile([128, C], f32, tag="o")
                for nh in range(2):
                    prps = pps.tile([128, 512], f32, tag="st")
                    for p in range(NPAIR):
                        nc.tensor.matmul(
                            prps[:], Y[:, p, tj * 128:(tj + 1) * 128],
                            wpT_r[:, p, nh * 512:(nh + 1) * 512],
                            start=(p == 0), stop=(p == NPAIR - 1))
                    nc.vector.tensor_copy(o_sb[:, nh * 512:(nh + 1) * 512], prps[:])
                row = j * 512 + tj * 128
                nc.sync.dma_start(out=out_d[row:row + 128, :], in_=o_sb[:])


def _get_nc(reps=1):
    key = f"nc{reps}"
    if key not in _CACHE:
        _CACHE[key] = _build_nc(reps)
    return _CACHE[key]


def make_in_maps(x, w_qkv, w_proj):
    """Shard full inputs into the 8 per-core input maps."""
    x = np.asarray(x, dtype=np.float32)
    w_qkv = np.asarray(w_qkv, dtype=np.float32)
    w_proj = np.asarray(w_proj, dtype=np.float32)
    mask = np.where(np.arange(128)[:, None] <= np.arange(128)[None, :],
                    np.float32(0.0), np.float32(NEG)).astype(np.float32)
    in_maps = []
    for c in range(8):
        b, hg = c // 2, c % 2
        sl = slice(hg * CO_Q, (hg + 1) * CO_Q)
        in_maps.append({
            "xT": np.ascontiguousarray(x[b].T),
            "wqT": np.ascontiguousarray(w_qkv[0 * C:1 * C][sl].T),
            "wkT": np.ascontiguousarray(w_qkv[1 * C:2 * C][sl].T),
            "wvT": np.ascontiguousarray(w_qkv[2 * C:3 * C][sl].T),
            "wpT": np.ascontiguousarray(w_proj[:, sl].T),
            "mask": mask,
        })
    return in_maps


def gather(results):
    """Sum the two head-group partials per batch, stack batches."""
    out = np.empty((B, T, C), dtype=np.float32)
    for b in range(B):
        out[b] = results[2 * b]["out"] + results[2 * b + 1]["out"]
    return out


def kernel(x, w_qkv, w_proj):
    nc = _get_nc()
    in_maps = make_in_maps(x, w_qkv, w_proj)
    res = run_bass_kernel_spmd(nc, in_maps, core_ids=list(range(8)))
    return gather(res.results)

